# revision 3
# baseline (speedup 1.0000x reference)
"""Data-adaptive weight-ensembling MLP (per-sample expert-merged FFN) on 8 trn2 cores.

Math (per sample b):
  c[b,:,:]  = gate(x)[b].reshape(E, L)          (2-layer relu MLP gate)
  W1[b] = bW1 + sum_e c[b,e,0] tvW1[e];  b1[b] = bb1 + sum_e c[b,e,1] tvb1[e]
  W2[b] = bW2 + sum_e c[b,e,2] tvW2[e];  b2[b] = bb2 + sum_e c[b,e,3] tvb2[e]
  out[b] = relu(x[b] @ W1[b].T + b1[b]) @ W2[b].T + b2[b]

Merged weights are never materialized:
  x[b] @ W1[b].T = x[b] @ bW1.T + sum_e c[b,e,0] (x[b] @ tvW1[e].T)
and the weighted expert sum happens inside PSUM accumulation: for expert e the
matmul stationary operand is X1T[e][d, b] = x[b, d] * c[b, e, 0], so every
task-vector element streams through the PE exactly once.

Sharding (8 cores): DFF=4096 split into 8 slices of 512. Core k computes
layer-1 output columns in its slice (full d-contraction locally -> exact
pre-activation -> local relu), then contracts layer 2 over the same f-slice.
One final AllReduce ([16,1024] = 64KB) sums layer-2 partials. Task-vector
banks are sharded along DFF (64MB/core); gate weights replicated.
"""

import contextlib

import numpy as np

B, D, DFF, E, L = 16, 1024, 4096, 16, 4
NCORES = 8
OSL = DFF // NCORES          # 512: per-core DFF slice
KC1 = D // 128               # 8 k-chunks for the d contraction
KC2 = OSL // 128             # 4 k-chunks for the f contraction

_cache = {}


def _build():
    import concourse.bacc as bacc
    import concourse.bass as bass
    import concourse.tile as tile
    import concourse.mybir as mybir
    from concourse.masks import make_identity

    f32 = mybir.dt.float32
    Relu = mybir.ActivationFunctionType.Relu
    nc = bacc.Bacc("TRN2", target_bir_lowering=False, debug=False,
                   num_devices=NCORES)

    # ---- I/O (per-core data supplied via in_maps) ----
    xT_h = nc.dram_tensor("xT", [128, KC1, B], f32, kind="ExternalInput")
    gw1_h = nc.dram_tensor("gw1", [128, KC1, D], f32, kind="ExternalInput")
    gb1_h = nc.dram_tensor("gb1v", [1, D], f32, kind="ExternalInput")
    gw2_h = nc.dram_tensor("gw2", [128, KC1, E * L], f32, kind="ExternalInput")
    gb2_h = nc.dram_tensor("gb2v", [1, E * L], f32, kind="ExternalInput")
    tv1_h = nc.dram_tensor("tv1", [E, 128, KC1, OSL], f32, kind="ExternalInput")
    bw1_h = nc.dram_tensor("bw1", [128, KC1, OSL], f32, kind="ExternalInput")
    bb1_h = nc.dram_tensor("bb1v", [1, OSL], f32, kind="ExternalInput")
    tvb1_h = nc.dram_tensor("tvb1", [E, OSL], f32, kind="ExternalInput")
    tv2_h = nc.dram_tensor("tv2", [E, 128, KC2, D], f32, kind="ExternalInput")
    bw2_h = nc.dram_tensor("bw2", [128, KC2, D], f32, kind="ExternalInput")
    bb2_h = nc.dram_tensor("bb2v", [1, D], f32, kind="ExternalInput")
    tvb2_h = nc.dram_tensor("tvb2", [E, D], f32, kind="ExternalInput")
    out_h = nc.dram_tensor("out", [B, D], f32, kind="ExternalOutput")

    ar_in = nc.dram_tensor("ar_in", [B, D], f32, kind="Internal")
    ar_out = nc.dram_tensor("ar_out", [B, D], f32, kind="Internal",
                            addr_space="Shared")

    with tile.TileContext(nc) as tc, contextlib.ExitStack() as ctx:
        const = ctx.enter_context(tc.tile_pool(name="const", bufs=1))
        small = ctx.enter_context(tc.tile_pool(name="small", bufs=1))
        gwp = ctx.enter_context(tc.tile_pool(name="gwp", bufs=1))
        tvp1 = ctx.enter_context(tc.tile_pool(name="tvp1", bufs=3))
        tvp2 = ctx.enter_context(tc.tile_pool(name="tvp2", bufs=3))
        pacc = ctx.enter_context(tc.tile_pool(name="pacc", bufs=1,
                                              space="PSUM"))
        psml = ctx.enter_context(tc.tile_pool(name="psml", bufs=2,
                                              space="PSUM"))

        # constants
        ones1 = const.tile([1, B], f32)
        nc.vector.memset(ones1[:], 1.0)
        ident16 = const.tile([B, B], f32)
        make_identity(nc, ident16[:])
        ones16_128 = const.tile([B, 128], f32)
        nc.vector.memset(ones16_128[:], 1.0)

        # small inputs
        xT = small.tile([128, KC1, B], f32)
        nc.sync.dma_start(out=xT[:], in_=xT_h.ap())
        gb1v = small.tile([1, D], f32)
        nc.sync.dma_start(out=gb1v[:], in_=gb1_h.ap())
        gb2v = small.tile([1, E * L], f32)
        nc.sync.dma_start(out=gb2v[:], in_=gb2_h.ap())
        bb1v = small.tile([1, OSL], f32)
        nc.sync.dma_start(out=bb1v[:], in_=bb1_h.ap())
        tvb1t = small.tile([E, OSL], f32)
        nc.sync.dma_start(out=tvb1t[:], in_=tvb1_h.ap())
        bb2v = small.tile([1, D], f32)
        nc.sync.dma_start(out=bb2v[:], in_=bb2_h.ap())
        tvb2t = small.tile([E, D], f32)
        nc.sync.dma_start(out=tvb2t[:], in_=tvb2_h.ap())
        gw2t = small.tile([128, KC1, E * L], f32)
        nc.sync.dma_start(out=gw2t[:], in_=gw2_h.ap())
        gw1t = gwp.tile([128, KC1, D], f32)
        nc.sync.dma_start(out=gw1t[:], in_=gw1_h.ap())

        # ---- gate layer 1: g_h = relu(x @ gW1.T + gb1) ----
        g_h = small.tile([B, D], f32)
        for n in range(2):
            gps = pacc.tile([B, 512], f32, tag="gps")
            nc.tensor.matmul(gps[:], ones1[:], gb1v[:, n * 512:(n + 1) * 512],
                             start=True, stop=False)
            for kc in range(KC1):
                nc.tensor.matmul(gps[:], xT[:, kc, :],
                                 gw1t[:, kc, n * 512:(n + 1) * 512],
                                 start=False, stop=(kc == KC1 - 1))
            nc.scalar.activation(g_h[:, n * 512:(n + 1) * 512], gps[:], Relu)

        # ---- transpose g_h -> ghT [128, (kc, b)] ----
        ghT = small.tile([128, KC1, B], f32)
        for kc in range(KC1):
            pt = psml.tile([128, B], f32, tag="ps")
            nc.tensor.transpose(pt[:], g_h[:, kc * 128:(kc + 1) * 128],
                                ident16[:])
            nc.vector.tensor_copy(ghT[:, kc, :], pt[:])

        # ---- gate layer 2: codings = g_h @ gW2.T + gb2;  cod[b, e, l] ----
        cps = psml.tile([B, E * L], f32, tag="ps")
        nc.tensor.matmul(cps[:], ones1[:], gb2v[:], start=True, stop=False)
        for kc in range(KC1):
            nc.tensor.matmul(cps[:], ghT[:, kc, :], gw2t[:, kc, :],
                             start=False, stop=(kc == KC1 - 1))
        cod = small.tile([B, E, L], f32)
        nc.vector.tensor_copy(cod[:], cps[:].rearrange("b (e l) -> b e l", e=E))

        # ---- bias-coefficient matrices cT_l[e, b] = c[b, e, l], l=1,3 ----
        cT = {}
        for l in (1, 3):
            cl = small.tile([B, E], f32, name=f"cl{l}", tag=f"cl{l}")
            nc.vector.tensor_copy(cl[:], cod[:, :, l])
            ptc = psml.tile([B, E], f32, tag="ps")
            nc.tensor.transpose(ptc[:], cl[:], ident16[:])
            cTl = small.tile([E, B], f32, name=f"cT{l}", tag=f"cT{l}")
            nc.vector.tensor_copy(cTl[:], ptc[:])
            cT[l] = cTl

        # ---- broadcast tiles cbc[l][e][p, b] = c[b, e, l], l=0,2 ----
        # via PE: ones[16,128].T @ diag(c[:,e,l])
        cbc = {0: [], 2: []}
        for l in (0, 2):
            for e in range(E):
                diag = small.tile([B, B], f32, name=f"dg{l}_{e}", tag="diag")
                nc.vector.tensor_scalar_mul(diag[:], ident16[:],
                                            cod[:, e, l:l + 1])
                pb = psml.tile([128, B], f32, tag="ps")
                nc.tensor.matmul(pb[:], ones16_128[:], diag[:],
                                 start=True, stop=True)
                bc = small.tile([128, B], f32, name=f"bc{l}_{e}",
                                tag=f"bc{l}_{e}")
                nc.vector.tensor_copy(bc[:], pb[:])
                cbc[l].append(bc)

        # ---- X1T[e][128, kc, b] = xT * c1[b, e] ----
        x1t = []
        for e in range(E):
            t = small.tile([128, KC1, B], f32, name=f"x1t{e}", tag=f"x1t{e}")
            nc.vector.tensor_mul(
                t[:], xT[:],
                cbc[0][e][:, None, :].broadcast_to([128, KC1, B]))
            x1t.append(t)

        # ---- layer 1: psum1[b, o] = full local pre-activation ----
        psum1 = pacc.tile([B, OSL], f32, tag="psum1")
        nc.tensor.matmul(psum1[:], ones1[:], bb1v[:], start=True, stop=False)
        nc.tensor.matmul(psum1[:], cT[1][:], tvb1t[:], start=False, stop=False)
        for e in range(E + 1):
            tvt = tvp1.tile([128, KC1, OSL], f32, tag="tvt1")
            nc.sync.dma_start(out=tvt[:],
                              in_=bw1_h.ap() if e == E else tv1_h.ap()[e])
            lhs = xT if e == E else x1t[e]
            for kc in range(KC1):
                nc.tensor.matmul(psum1[:], lhs[:, kc, :], tvt[:, kc, :],
                                 start=False,
                                 stop=(e == E and kc == KC1 - 1))

        h1 = small.tile([B, OSL], f32)
        nc.scalar.activation(h1[:], psum1[:], Relu)

        # ---- transpose h1 -> h1T [128, (fc, b)] ----
        h1T = small.tile([128, KC2, B], f32)
        for fc in range(KC2):
            pt2 = psml.tile([128, B], f32, tag="ps")
            nc.tensor.transpose(pt2[:], h1[:, fc * 128:(fc + 1) * 128],
                                ident16[:])
            nc.vector.tensor_copy(h1T[:, fc, :], pt2[:])

        # ---- X2T[e][128, fc, b] = h1T * c2[b, e] ----
        x2t = []
        for e in range(E):
            t = small.tile([128, KC2, B], f32, name=f"x2t{e}", tag=f"x2t{e}")
            nc.vector.tensor_mul(
                t[:], h1T[:],
                cbc[2][e][:, None, :].broadcast_to([128, KC2, B]))
            x2t.append(t)

        # ---- layer 2: psum2[n][b, j] partial over local f-slice ----
        psum2 = []
        for n in range(2):
            p = pacc.tile([B, 512], f32, tag=f"psum2_{n}")
            nc.tensor.matmul(p[:], ones1[:], bb2v[:, n * 512:(n + 1) * 512],
                             start=True, stop=False)
            nc.tensor.matmul(p[:], cT[3][:], tvb2t[:, n * 512:(n + 1) * 512],
                             start=False, stop=False)
            psum2.append(p)
        for e in range(E + 1):
            tvt2 = tvp2.tile([128, KC2, D], f32, tag="tvt2")
            nc.sync.dma_start(out=tvt2[:],
                              in_=bw2_h.ap() if e == E else tv2_h.ap()[e])
            lhs = h1T if e == E else x2t[e]
            for fc in range(KC2):
                for n in range(2):
                    nc.tensor.matmul(psum2[n][:], lhs[:, fc, :],
                                     tvt2[:, fc, n * 512:(n + 1) * 512],
                                     start=False,
                                     stop=(e == E and fc == KC2 - 1))

        outp = small.tile([B, D], f32)
        for n in range(2):
            nc.vector.tensor_copy(outp[:, n * 512:(n + 1) * 512], psum2[n][:])

        # ---- final AllReduce over all 8 cores ----
        nc.sync.dma_start(out=ar_in.ap(), in_=outp[:])
        nc.gpsimd.collective_compute(
            "AllReduce", mybir.AluOpType.add,
            replica_groups=[list(range(NCORES))],
            ins=[ar_in.ap().opt()],
            outs=[ar_out.ap().opt()],
        )
        nc.sync.dma_start(out=out_h.ap(), in_=ar_out.ap())

    nc.compile()
    return nc


def _prep_inputs(x, gW1, gb1, gW2, gb2, bW1, bb1, bW2, bb2,
                 tvW1, tvb1, tvW2, tvb2):
    """Build the 8 per-core in_maps (all float32, DMA-friendly layouts)."""
    f = np.float32
    asf = lambda a: np.ascontiguousarray(a, dtype=f)

    xT = asf(x.T.reshape(KC1, 128, B).transpose(1, 0, 2))
    gw1 = asf(gW1.T.reshape(KC1, 128, D).transpose(1, 0, 2))
    gw2 = asf(gW2.T.reshape(KC1, 128, E * L).transpose(1, 0, 2))
    gb1v = asf(gb1.reshape(1, D))
    gb2v = asf(gb2.reshape(1, E * L))

    in_maps = []
    for k in range(NCORES):
        o0 = k * OSL
        tv1 = asf(tvW1[:, o0:o0 + OSL, :].transpose(0, 2, 1)
                  .reshape(E, KC1, 128, OSL).transpose(0, 2, 1, 3))
        bw1 = asf(bW1[o0:o0 + OSL, :].T.reshape(KC1, 128, OSL)
                  .transpose(1, 0, 2))
        tv2 = asf(tvW2[:, :, o0:o0 + OSL].transpose(0, 2, 1)
                  .reshape(E, KC2, 128, D).transpose(0, 2, 1, 3))
        bw2 = asf(bW2[:, o0:o0 + OSL].T.reshape(KC2, 128, D)
                  .transpose(1, 0, 2))
        zero = k != 0
        in_maps.append(dict(
            xT=xT, gw1=gw1, gb1v=gb1v, gw2=gw2, gb2v=gb2v,
            tv1=tv1, bw1=bw1,
            bb1v=asf(bb1[o0:o0 + OSL].reshape(1, OSL)),
            tvb1=asf(tvb1[:, o0:o0 + OSL]),
            tv2=tv2, bw2=bw2,
            bb2v=np.zeros((1, D), f) if zero else asf(bb2.reshape(1, D)),
            tvb2=np.zeros((E, D), f) if zero else asf(tvb2),
        ))
    return in_maps


def kernel(**inputs):
    from concourse.bass_utils import run_bass_kernel_spmd

    if "nc" not in _cache:
        _cache["nc"] = _build()
    nc = _cache["nc"]

    in_maps = _prep_inputs(**{k: np.asarray(v) for k, v in inputs.items()})
    res = run_bass_kernel_spmd(nc, in_maps, core_ids=list(range(NCORES)))
    return res.results[0]["out"]
